# revision 13
# baseline (speedup 1.0000x reference)
"""Spiking self-attention (SpikFormer-style SSA block) on 8 TRN2 NeuronCores.

Sharding: data-parallel over B (B=8 -> 1 batch element per core).
BN batch stats are all-reduced across cores (2 AllReduces of channel stats).

Precision strategy (the problem is chaotic -- LIF thresholds have margins
down to ~1e-8, so matmuls must be fp32-quality):
  - LIF recurrences: fp32, bit-exact op ordering vs the jax reference.
  - qkv / proj matmuls: weights split into fp16 hi + fp16 lo*4096 (weight
    repr error ~7e-9 = fp32 ulp), spikes are exact in fp16; two 1-cyc/row
    PE passes + fp32 PSUM accumulation == fp32-quality matmul at 2x bf16 cost.
  - attention (k^T v, q @ kv): pure fp16 spike/int matmuls -- exact integer
    arithmetic (counts <= 196 exact in fp16, psum fp32).
  - BN: fp32 stats, rsqrt via DVE reciprocal + ACT sqrt + 2 Newton steps.
"""

import math
import numpy as np

import concourse.bass as bass
import concourse.bacc as bacc
import concourse.tile as tile
from concourse import mybir, masks
from concourse import bass_utils
from concourse.mybir import AluOpType as op
from concourse.mybir import ActivationFunctionType as act

F32 = mybir.dt.float32
F16 = mybir.dt.float16

T, B, N, C = 16, 8, 196, 512
H, HD = 8, 64
O3 = 3 * C            # 1536
NT0, NT1 = 128, N - 128   # 128 + 68 token split
KT = C // 128         # 4 k-tiles
OT1 = O3 // 128       # 12 o-tiles (qkv)
OT2 = C // 128        # 4 o-tiles (proj)
NB = B * N            # 1568 elements per BN channel
EPS = 1e-5
LO_SCALE = 4096.0

N_CORES = 8


def _build(sg_in, sg_q, sg_k, sg_v, sg_proj, b1_zero=True):
    """Build the Bass program (SPMD, one batch element per core)."""
    nc = bacc.Bacc("TRN2", target_bir_lowering=False, debug=False,
                   num_devices=N_CORES)

    # ---- I/O ----
    x_d = nc.dram_tensor("x", [T, N, C], F32, kind="ExternalInput").ap()
    w1h_d = nc.dram_tensor("w1h", [C, O3], F16, kind="ExternalInput").ap()
    w1l_d = nc.dram_tensor("w1l", [C, O3], F16, kind="ExternalInput").ap()
    w2h_d = nc.dram_tensor("w2h", [C, C], F16, kind="ExternalInput").ap()
    w2l_d = nc.dram_tensor("w2l", [C, C], F16, kind="ExternalInput").ap()
    qb_d = nc.dram_tensor("qb", [128, OT1], F32, kind="ExternalInput").ap()
    pb_d = nc.dram_tensor("pb", [128, OT2], F32, kind="ExternalInput").ap()
    g1_d = nc.dram_tensor("g1", [128, OT1 * T], F32, kind="ExternalInput").ap()
    b1_d = nc.dram_tensor("b1", [128, OT1 * T], F32, kind="ExternalInput").ap()
    g2_d = nc.dram_tensor("g2", [128, OT2 * T], F32, kind="ExternalInput").ap()
    b2_d = nc.dram_tensor("b2", [128, OT2 * T], F32, kind="ExternalInput").ap()
    out_d = nc.dram_tensor("out", [T, N, C], F32, kind="ExternalOutput").ap()

    with tile.TileContext(nc) as tc:
        import contextlib
        stack = contextlib.ExitStack()
        const = stack.enter_context(tc.tile_pool(name="const", bufs=1))
        state = stack.enter_context(tc.tile_pool(name="state", bufs=1))
        work = stack.enter_context(tc.tile_pool(name="work", bufs=3))
        spk = stack.enter_context(tc.tile_pool(name="spk", bufs=2))
        psum = stack.enter_context(tc.tile_pool(name="psum", bufs=2, space="PSUM"))
        dram = stack.enter_context(tc.tile_pool(name="dram", bufs=1, space="DRAM"))

        # ---- constants ----
        ident = const.tile([128, 128], F16, tag="identf16", name="identf16")
        masks.make_identity(nc, ident[:])
        identf = const.tile([128, 128], F32, tag="identf32", name="identf32")
        masks.make_identity(nc, identf[:])

        w1h = [const.tile([128, O3], F16, tag=f"w1h{k}", name=f"w1h{k}") for k in range(KT)]
        w1l = [const.tile([128, O3], F16, tag=f"w1l{k}", name=f"w1l{k}") for k in range(KT)]
        w2h = [const.tile([128, C], F16, tag=f"w2h{k}", name=f"w2h{k}") for k in range(KT)]
        w2l = [const.tile([128, C], F16, tag=f"w2l{k}", name=f"w2l{k}") for k in range(KT)]
        for k in range(KT):
            nc.sync.dma_start(w1h[k][:], w1h_d[k * 128:(k + 1) * 128, :])
            nc.sync.dma_start(w1l[k][:], w1l_d[k * 128:(k + 1) * 128, :])
            nc.sync.dma_start(w2h[k][:], w2h_d[k * 128:(k + 1) * 128, :])
            nc.sync.dma_start(w2l[k][:], w2l_d[k * 128:(k + 1) * 128, :])

        qb = const.tile([128, OT1], F32, tag="qb", name="qb")
        pb = const.tile([128, OT2], F32, tag="pb", name="pb")
        g1 = const.tile([128, OT1 * T], F32, tag="g1", name="g1")
        b1 = const.tile([128, OT1 * T], F32, tag="b1", name="b1")
        g2 = const.tile([128, OT2 * T], F32, tag="g2", name="g2")
        b2 = const.tile([128, OT2 * T], F32, tag="b2", name="b2")
        for t_ap, d_ap in [(qb, qb_d), (pb, pb_d), (g1, g1_d), (b1, b1_d),
                           (g2, g2_d), (b2, b2_d)]:
            nc.sync.dma_start(t_ap[:], d_ap[:, :])

        # ---- states ----
        v1 = [state.tile([128, C], F32, tag=f"v1_{i}", name=f"v1_{i}") for i in range(2)]
        v2 = state.tile([128, OT1 * N], F32, tag="v2", name="v2")
        v4 = state.tile([128, OT2 * N], F32, tag="v4", name="v4")
        st1 = state.tile([128, 2 * OT1 * T], F32, tag="st1", name="st1")
        st2 = state.tile([128, 2 * OT2 * T], F32, tag="st2", name="st2")
        for s in (v1[0], v1[1], v2, v4, st1, st2):
            nc.vector.memset(s[:], 0.0)

        # DRAM scratch
        qkv_dr = dram.tile([T, OT1, 128, N], F32, tag="qkv_dr", name="qkv_dr")
        po_dr = dram.tile([T, OT2, 128, N], F32, tag="po_dr", name="po_dr")
        st1_in = dram.tile([128, 2 * OT1 * T], F32, tag="st1_in", name="st1_in")
        st1_out = dram.tile([128, 2 * OT1 * T], F32, tag="st1_out", name="st1_out")
        st2_in = dram.tile([128, 2 * OT2 * T], F32, tag="st2_in", name="st2_in")
        st2_out = dram.tile([128, 2 * OT2 * T], F32, tag="st2_out", name="st2_out")

        nsl = [(0, NT0), (NT0, NT1)]  # (offset, size) token tiles

        # ================= Phase A: LIF1 + QKV matmul + bn1 stats ==========
        for t in range(T):
            # load x_t  [196, 512]
            xs = [work.tile([128, C], F32, tag=f"x{i}", name=f"x{i}") for i in range(2)]
            for i, (o, sz) in enumerate(nsl):
                nc.sync.dma_start(xs[i][:sz, :], x_d[t, o:o + sz, :])

            # LIF1 (bit-exact): d = x - v; vp = d*sg + v; s = vp>=.5; v = (vp<.5)*vp
            s1 = [spk.tile([128, C], F16, tag=f"s1_{i}", name=f"s1_{i}") for i in range(2)]
            for i, (o, sz) in enumerate(nsl):
                d = work.tile([128, C], F32, tag="lif1_d", name="lif1_d")
                vp = work.tile([128, C], F32, tag="lif1_vp", name="lif1_vp")
                nc.vector.tensor_tensor(d[:sz, :], xs[i][:sz, :], v1[i][:sz, :], op.subtract)
                nc.vector.scalar_tensor_tensor(vp[:sz, :], d[:sz, :], sg_in, v1[i][:sz, :], op.mult, op.add)
                nc.vector.tensor_scalar(s1[i][:sz, :], vp[:sz, :], 0.5, None, op.is_ge)
                nc.vector.scalar_tensor_tensor(v1[i][:sz, :], vp[:sz, :], 0.5, vp[:sz, :], op.is_lt, op.mult)

            # transpose spikes -> s1T [512(4 tiles of 128), 196]
            s1t = spk.tile([128, KT * N], F16, tag="s1t", name="s1t")
            for ct in range(KT):
                for i, (o, sz) in enumerate(nsl):
                    tp = psum.tile([128, 128], F16, tag="tp", name="tp")
                    nc.tensor.transpose(tp[:128, :sz], s1[i][:sz, ct * 128:(ct + 1) * 128],
                                        ident[:sz, :sz])
                    nc.scalar.activation(s1t[:, ct * N + o: ct * N + o + sz],
                                         tp[:128, :sz], act.Copy)

            # QKV matmul: out^T [12 o-tiles of 128, 196]
            for ot in range(OT1):
                ph = psum.tile([128, N], F32, tag="mma", name="mma")
                pl = psum.tile([128, N], F32, tag="mmb", name="mmb")
                for k in range(KT):
                    nc.tensor.matmul(ph[:, :], w1h[k][:, ot * 128:(ot + 1) * 128],
                                     s1t[:, k * N:(k + 1) * N],
                                     start=(k == 0), stop=(k == KT - 1))
                for k in range(KT):
                    nc.tensor.matmul(pl[:, :], w1l[k][:, ot * 128:(ot + 1) * 128],
                                     s1t[:, k * N:(k + 1) * N],
                                     start=(k == 0), stop=(k == KT - 1))
                hi = work.tile([128, N], F32, tag="hi", name="hi")
                nc.scalar.activation(hi[:, :], ph[:, :], act.Identity,
                                     bias=qb[:, ot:ot + 1], scale=1.0)
                qk = work.tile([128, N], F32, tag="qk", name="qk")
                col = ot * T + t
                nc.vector.scalar_tensor_tensor(qk[:, :], pl[:, :], 1.0 / LO_SCALE,
                                               hi[:, :], op.mult, op.add,
                                               accum_out=st1[:, col:col + 1])
                sq = work.tile([128, N], F32, tag="sq", name="sq")
                nc.scalar.activation(sq[:, :], qk[:, :], act.Square,
                                     accum_out=st1[:, OT1 * T + col: OT1 * T + col + 1])
                nc.sync.dma_start(qkv_dr[t, ot], qk[:, :])

        # ================= AllReduce bn1 stats ==========
        nc.sync.dma_start(st1_in[:], st1[:])
        nc.gpsimd.collective_compute(
            "AllReduce", op.add,
            ins=[st1_in.opt()], outs=[st1_out.opt()],
            replica_groups=[list(range(N_CORES))],
        )
        g1sum = const.tile([128, 2 * OT1 * T], F32, tag="g1sum", name="g1sum")
        nc.sync.dma_start(g1sum[:], st1_out[:])

        # ---- bn1 params: scale_h = 0.5*rstd*g ; bias_h = 0.5*(beta - mean*rstd*g)
        def bn_params(gsum, n_ch, g_t, b_t, halve):
            nch = n_ch  # number of (ot,t) columns
            mean = const.tile([128, nch], F32, tag=f"mean_{n_ch}", name=f"mean_{n_ch}")
            e2p = const.tile([128, nch], F32, tag=f"e2p_{n_ch}", name=f"e2p_{n_ch}")
            varp = const.tile([128, nch], F32, tag=f"varp_{n_ch}", name=f"varp_{n_ch}")
            rs = const.tile([128, nch], F32, tag=f"rs_{n_ch}", name=f"rs_{n_ch}")
            tmp = const.tile([128, nch], F32, tag=f"tmp_{n_ch}", name=f"tmp_{n_ch}")
            tmp2 = const.tile([128, nch], F32, tag=f"tmp2_{n_ch}", name=f"tmp2_{n_ch}")
            sc = const.tile([128, nch], F32, tag=f"sc_{n_ch}", name=f"sc_{n_ch}")
            bi = const.tile([128, nch], F32, tag=f"bi_{n_ch}", name=f"bi_{n_ch}")
            nc.vector.tensor_scalar(mean[:], gsum[:, 0:nch], 1.0 / NB, None, op.mult)
            nc.vector.tensor_scalar(e2p[:], gsum[:, nch:2 * nch], 1.0 / NB, EPS, op.mult, op.add)
            nc.vector.tensor_tensor(tmp[:], mean[:], mean[:], op.mult)
            nc.vector.scalar_tensor_tensor(varp[:], tmp[:], -1.0, e2p[:], op.mult, op.add)
            nc.vector.reciprocal(tmp[:], varp[:])
            nc.scalar.activation(rs[:], tmp[:], act.Sqrt)
            # 2 Newton iterations: rs *= (1.5 - 0.5*varp*rs^2)
            for _ in range(2):
                nc.vector.tensor_tensor(tmp[:], rs[:], rs[:], op.mult)
                nc.vector.tensor_tensor(tmp2[:], tmp[:], varp[:], op.mult)
                nc.vector.tensor_scalar(tmp[:], tmp2[:], -0.5, 1.5, op.mult, op.add)
                nc.vector.tensor_tensor(rs[:], rs[:], tmp[:], op.mult)
            # rg = rs*g ; sc[slice] = h*rg ; bias[slice] = h*(beta - mean*rg)
            # h is the per-column-group LIF input gain (sg of the LIF fed by
            # this BN output; folds the v' = (1-sg)*v + sg*y update).
            nc.vector.tensor_tensor(tmp[:], rs[:], g_t[:], op.mult)
            nc.vector.tensor_tensor(tmp2[:], mean[:], tmp[:], op.mult)
            nc.vector.tensor_tensor(e2p[:], b_t[:], tmp2[:], op.subtract)
            for lo, hi_, h in halve:
                nc.vector.tensor_scalar(sc[:, lo:hi_], tmp[:, lo:hi_], h, None, op.mult)
                nc.vector.tensor_scalar(bi[:, lo:hi_], e2p[:, lo:hi_], h, None, op.mult)
            return sc, bi, mean

        sc1, bi1, mu1 = bn_params(g1sum, OT1 * T, g1, b1,
                                  [(0, 4 * T, sg_q), (4 * T, 8 * T, sg_k),
                                   (8 * T, 12 * T, sg_v)])

        # ================= Phase B: bn1-norm + LIF qkv + attention + LIF proj
        #                   + proj matmul + bn2 stats ==========
        for t in range(T):
            sT = spk.tile([128, OT1 * N], F16, tag="sT", name="sT")
            for ot in range(OT1):
                qk = work.tile([128, N], F32, tag="qk_b", name="qk_b")
                nc.sync.dma_start(qk[:, :], qkv_dr[t, ot])
                col = ot * T + t
                ys = work.tile([128, N], F32, tag="ys", name="ys")
                if b1_zero:
                    # ys = (x - mu) * (rstd*g*sg): cancellation-free normalize
                    nc.vector.tensor_scalar(ys[:, :], qk[:, :], mu1[:, col:col + 1],
                                            sc1[:, col:col + 1], op.subtract, op.mult)
                else:
                    nc.scalar.activation(ys[:, :], qk[:, :], act.Identity,
                                         bias=bi1[:, col:col + 1],
                                         scale=sc1[:, col:col + 1])
                # LIF (folded): vp = 0.5*v + ys ; s = vp>=.5 ; v = (vp<.5)*vp
                vsl = v2[:, ot * N:(ot + 1) * N]
                vp = work.tile([128, N], F32, tag="vp_b", name="vp_b")
                one_m_sg = 1.0 - (sg_q if ot < 4 else (sg_k if ot < 8 else sg_v))
                nc.vector.scalar_tensor_tensor(vp[:, :], vsl, one_m_sg, ys[:, :], op.mult, op.add)
                nc.vector.tensor_scalar(sT[:, ot * N:(ot + 1) * N], vp[:, :], 0.5, None, op.is_ge)
                nc.vector.scalar_tensor_tensor(vsl, vp[:, :], 0.5, vp[:, :], op.is_lt, op.mult)

            # transpose k/v spikes -> [196, 512] layout (2 token tiles)
            kT = [spk.tile([128, C], F16, tag=f"kT{i}", name=f"kT{i}") for i in range(2)]
            vT = [spk.tile([128, C], F16, tag=f"vT{i}", name=f"vT{i}") for i in range(2)]
            for j, dst in [(1, kT), (2, vT)]:
                for ci in range(4):
                    otg = 4 * j + ci
                    for i, (o, sz) in enumerate(nsl):
                        tp = psum.tile([128, 128], F16, tag="tp", name="tp")
                        nc.tensor.transpose(tp[:sz, :128],
                                            sT[:, otg * N + o: otg * N + o + sz],
                                            ident[:128, :128])
                        nc.scalar.activation(dst[i][:sz, ci * 128:(ci + 1) * 128],
                                             tp[:sz, :128], act.Copy)

            # attention per head-pair
            os_ = work.tile([128, OT2 * N], F32, tag="os", name="os")
            for ct in range(4):  # head pair (2*ct, 2*ct+1)
                kvp = psum.tile([128, HD], F32, tag="kvp", name="kvp")
                for hh in range(2):
                    h = 2 * ct + hh
                    off = hh * 64
                    hc = h * 64
                    nc.tensor.matmul(kvp[off:off + 64, :],
                                     kT[0][:, hc:hc + 64], vT[0][:, hc:hc + 64],
                                     start=True, stop=False,
                                     tile_position=(0, off))
                    nc.tensor.matmul(kvp[off:off + 64, :],
                                     kT[1][:NT1, hc:hc + 64], vT[1][:NT1, hc:hc + 64],
                                     start=False, stop=True,
                                     tile_position=(0, off))
                kv = work.tile([128, HD], F16, tag="kv", name="kv")
                nc.scalar.activation(kv[:, :], kvp[:, :], act.Copy)
                outp = psum.tile([128, N], F32, tag="mma", name="mma")
                for hh in range(2):
                    off = hh * 64
                    nc.tensor.matmul(outp[off:off + 64, :],
                                     kv[off:off + 64, :],
                                     sT[off:off + 64, ct * N:(ct + 1) * N],
                                     start=True, stop=True,
                                     tile_position=(off, off))
                nc.scalar.activation(os_[:, ct * N:(ct + 1) * N], outp[:, :], act.Copy)

            # LIF proj on V=8*v scale (threshold 4.0), bit-exact
            d4 = work.tile([128, OT2 * N], F32, tag="d4", name="d4")
            vp4 = work.tile([128, OT2 * N], F32, tag="vp4", name="vp4")
            spT = spk.tile([128, OT2 * N], F16, tag="spT", name="spT")
            nc.vector.tensor_tensor(d4[:], os_[:], v4[:], op.subtract)
            nc.vector.scalar_tensor_tensor(vp4[:], d4[:], sg_proj, v4[:], op.mult, op.add)
            nc.vector.tensor_scalar(spT[:], vp4[:], 4.0, None, op.is_ge)
            nc.vector.scalar_tensor_tensor(v4[:], vp4[:], 4.0, vp4[:], op.is_lt, op.mult)

            # proj matmul
            for ot in range(OT2):
                ph = psum.tile([128, N], F32, tag="mma", name="mma")
                pl = psum.tile([128, N], F32, tag="mmb", name="mmb")
                for k in range(KT):
                    nc.tensor.matmul(ph[:, :], w2h[k][:, ot * 128:(ot + 1) * 128],
                                     spT[:, k * N:(k + 1) * N],
                                     start=(k == 0), stop=(k == KT - 1))
                for k in range(KT):
                    nc.tensor.matmul(pl[:, :], w2l[k][:, ot * 128:(ot + 1) * 128],
                                     spT[:, k * N:(k + 1) * N],
                                     start=(k == 0), stop=(k == KT - 1))
                hi = work.tile([128, N], F32, tag="hi2", name="hi2")
                nc.scalar.activation(hi[:, :], ph[:, :], act.Identity,
                                     bias=pb[:, ot:ot + 1], scale=1.0)
                po = work.tile([128, N], F32, tag="po", name="po")
                col = ot * T + t
                nc.vector.scalar_tensor_tensor(po[:, :], pl[:, :], 1.0 / LO_SCALE,
                                               hi[:, :], op.mult, op.add,
                                               accum_out=st2[:, col:col + 1])
                sq = work.tile([128, N], F32, tag="sq2", name="sq2")
                nc.scalar.activation(sq[:, :], po[:, :], act.Square,
                                     accum_out=st2[:, OT2 * T + col: OT2 * T + col + 1])
                nc.sync.dma_start(po_dr[t, ot], po[:, :])

        # ================= AllReduce bn2 stats ==========
        nc.sync.dma_start(st2_in[:], st2[:])
        nc.gpsimd.collective_compute(
            "AllReduce", op.add,
            ins=[st2_in.opt()], outs=[st2_out.opt()],
            replica_groups=[list(range(N_CORES))],
        )
        g2sum = const.tile([128, 2 * OT2 * T], F32, tag="g2sum", name="g2sum")
        nc.sync.dma_start(g2sum[:], st2_out[:])
        sc2, bi2, _mu2 = bn_params(g2sum, OT2 * T, g2, b2, [(0, OT2 * T, 1.0)])

        # ================= Phase C: bn2 norm + final transpose + store ======
        for t in range(T):
            fout = [work.tile([128, C], F32, tag=f"fout{i}", name=f"fout{i}") for i in range(2)]
            for ot in range(OT2):
                pc = work.tile([128, N], F32, tag="pc", name="pc")
                nc.sync.dma_start(pc[:, :], po_dr[t, ot])
                col = ot * T + t
                fin = work.tile([128, N], F32, tag="fin", name="fin")
                nc.scalar.activation(fin[:, :], pc[:, :], act.Identity,
                                     bias=bi2[:, col:col + 1], scale=sc2[:, col:col + 1])
                for i, (o, sz) in enumerate(nsl):
                    tpf = psum.tile([128, 128], F32, tag="tp", name="tp")
                    nc.tensor.transpose(tpf[:sz, :128], fin[:, o:o + sz],
                                        identf[:128, :128])
                    nc.scalar.activation(fout[i][:sz, ot * 128:(ot + 1) * 128],
                                         tpf[:sz, :128], act.Copy)
            for i, (o, sz) in enumerate(nsl):
                nc.sync.dma_start(out_d[t, o:o + sz, :], fout[i][:sz, :])

        stack.close()

    nc.compile()
    return nc


_CACHE = {}


def _sigmoid32(w):
    w = np.float32(w)
    return float(np.float32(1.0) / (np.float32(1.0) + np.exp(-w, dtype=np.float32)))


def _prep(inputs):
    qkv_w = np.asarray(inputs["qkv_w"], dtype=np.float32)
    proj_w = np.asarray(inputs["proj_w"], dtype=np.float32)
    w1t = np.ascontiguousarray(qkv_w.T)           # [512, 1536]
    w2t = np.ascontiguousarray(proj_w.T)          # [512, 512]
    w1h = w1t.astype(np.float16)
    w1l = ((w1t - w1h.astype(np.float32)) * np.float32(LO_SCALE)).astype(np.float16)
    w2h = w2t.astype(np.float16)
    w2l = ((w2t - w2h.astype(np.float32)) * np.float32(LO_SCALE)).astype(np.float16)

    qb = np.ascontiguousarray(np.asarray(inputs["qkv_b"], np.float32).reshape(OT1, 128).T)
    pb = np.ascontiguousarray(np.asarray(inputs["proj_b"], np.float32).reshape(OT2, 128).T)
    g1 = np.ascontiguousarray(np.asarray(inputs["bn1_g"], np.float32)
                              .reshape(T, OT1, 128).transpose(2, 1, 0).reshape(128, OT1 * T))
    b1 = np.ascontiguousarray(np.asarray(inputs["bn1_b"], np.float32)
                              .reshape(T, OT1, 128).transpose(2, 1, 0).reshape(128, OT1 * T))
    g2 = np.ascontiguousarray(np.asarray(inputs["bn2_g"], np.float32)
                              .reshape(T, OT2, 128).transpose(2, 1, 0).reshape(128, OT2 * T))
    b2 = np.ascontiguousarray(np.asarray(inputs["bn2_b"], np.float32)
                              .reshape(T, OT2, 128).transpose(2, 1, 0).reshape(128, OT2 * T))
    return dict(w1h=w1h, w1l=w1l, w2h=w2h, w2l=w2l, qb=qb, pb=pb,
                g1=g1, b1=b1, g2=g2, b2=b2)


def kernel(_trace=False, **inputs):
    sg = tuple(_sigmoid32(inputs[k]) for k in ("w_in", "w_q", "w_k", "w_v", "w_proj"))
    b1z = not np.any(np.asarray(inputs["bn1_b"]))
    key = sg + (b1z,)
    if key not in _CACHE:
        _CACHE[key] = _build(*sg, b1_zero=b1z)
    nc = _CACHE[key]

    shared = _prep(inputs)
    x = np.asarray(inputs["x"], dtype=np.float32)
    in_maps = []
    for b in range(N_CORES):
        m = dict(shared)
        m["x"] = np.ascontiguousarray(x[:, b])
        in_maps.append(m)

    res = bass_utils.run_bass_kernel_spmd(nc, in_maps, core_ids=list(range(N_CORES)),
                                          trace=_trace)
    out = np.stack([r["out"] for r in res.results], axis=1)  # [T, B, N, C]
    if _trace:
        return out, res
    return out
